# revision 42
# baseline (speedup 1.0000x reference)
"""Trainium2 Bass kernel for nn_Loss_50989851738757 (multi-term pose/chamfer loss).

Strategy: pure data parallel over batch (b=16 -> 2 samples per core, 8 cores).
Each core computes, for its 2 samples, per-sample partial sums of every loss
term; a ones-matmul collapses the partition dim and ~64 partial scalars per
core are shipped back.  Host does the final (tiny) means + weighted sum.

Key device-side tricks:
- every pairwise-distance matrix is produced directly in PSUM via an
  augmented K=5 matmul
    lhsT = [-2*a_x, -2*a_y, -2*a_z, selfbias_a, ones]
    rhs  = [ b_x,    b_y,    b_z,   ones,       selfbias_b]
  so psum[i,j] = ||a_i||^2 + ||b_j||^2 - 2 a_i.b_j  (+ mask*1e20 folded into
  the pts selfbias for the chamfer orientations that need masking).
- min(sqrt(x)) == sqrt(min(x)): only min-reduced columns get sqrt'ed.
- the masked d1 sums come from one cross-matmul mcols^T @ scols whose
  diagonal is the per-row-tile masked sum.
- this toolchain allows only ONE sync-wait per PE matmul, so every tile a
  matmul consumes gets an ACT-engine final writer (in-place copy checkpoint
  where needed) and every PSUM slot is released by an ACT reader.
"""

import sys

import numpy as np

sys.path.insert(0, "/opt/trn_rl_repo")

import concourse.bass as bass
import concourse.bacc as bacc
import concourse.mybir as mybir
from concourse.bass_utils import run_bass_kernel_spmd
from concourse.tile import TileContext

F32 = mybir.dt.float32
AF = mybir.ActivationFunctionType
ALU = mybir.AluOpType
AX = mybir.AxisListType

NCORES = 8
B = 16
S = 2          # samples per core
N = 1024       # n1 == n2
K = 96         # keypoints
T = 8          # 128-row tiles per sample
BIGSQ = 1e20   # mask fill, pre-sqrt (sqrt -> 1e10 like the reference BIG)
TH = 0.1
W = 48         # partial slots

# partial slot map (columns of the (128, W) partials tile)
SL_D2 = 16     # 16..31  d2 sums            (s*8 + t)
SL_CD = 32     # 32, 33  cd min-dist sums   (rows 0:96)
SL_DIV = 34    # 34, 35  sum of clipped diversity dist (rows 0:96)
SL_NOCS = 36   # 36, 37  smooth-l1 sums     (rows 0:96)
SL_ROT = 38    # 38, 39  rotation col-norm sums (rows 0:3)
SL_DELTA = 40  # 40, 41  delta norm sums    (rows 0:128)
SL_TRANS = 42  # translation norms, rows 0:2 (one per sample)
SL_SIZE = 43   # size norms, rows 0:2
SL_DIST = 44   # distillation norms, rows 0:2

_PROGRAM = None


def _build_program():
    nc = bacc.Bacc()

    lhsA_d = nc.dram_tensor("lhsA", [5, S * N], F32, kind="ExternalInput")
    rhsA_d = nc.dram_tensor("rhsA", [5, S * N], F32, kind="ExternalInput")
    lhsB_d = nc.dram_tensor("lhsB", [5, S * N], F32, kind="ExternalInput")
    rhsB_d = nc.dram_tensor("rhsB", [5, S * N], F32, kind="ExternalInput")
    rhsC_d = nc.dram_tensor("rhsC", [5, S * N], F32, kind="ExternalInput")
    lhsK_d = nc.dram_tensor("lhsK", [5, S * K], F32, kind="ExternalInput")
    rhsD_d = nc.dram_tensor("rhsD", [5, S * K], F32, kind="ExternalInput")
    mcol = nc.dram_tensor("mcol", [128, S * T], F32, kind="ExternalInput")
    delta2 = nc.dram_tensor("delta2", [128, S * 24], F32, kind="ExternalInput")
    nocsP = nc.dram_tensor("nocsP", [K, S * 3], F32, kind="ExternalInput")
    smallA = nc.dram_tensor("smallA", [4, 24], F32, kind="ExternalInput")
    smallB = nc.dram_tensor("smallB", [S, 1036], F32, kind="ExternalInput")
    eye96 = nc.dram_tensor("eye96", [K, K], F32, kind="ExternalInput")
    mbig_d = nc.dram_tensor("mbig", [1, S * N], F32, kind="ExternalInput")
    consts = nc.dram_tensor("consts", [128, 97], F32, kind="ExternalInput")
    out = nc.dram_tensor("out", [W, 1], F32, kind="ExternalOutput")
    out2 = nc.dram_tensor("out2", [S * T, S * T], F32, kind="ExternalOutput")

    with TileContext(nc) as tc:
        with (
            tc.tile_pool(name="const", bufs=1) as cp,
            tc.tile_pool(name="work", bufs=3) as wp,
            tc.tile_pool(name="psbig", bufs=2, space="PSUM") as psbig,
            tc.tile_pool(name="psaux", bufs=3, space="PSUM") as psaux,
            tc.tile_pool(name="psfin", bufs=1, space="PSUM") as psfin,
        ):
            # ---- load inputs (one DMA per operand tile) ----
            lhsA = cp.tile([5, S * N], F32)   # [-2 pT, (pnorm2+mbig)*, ones]
            nc.gpsimd.dma_start(lhsA[:], lhsA_d[:])
            rhsA = cp.tile([5, S * N], F32)   # [rT, ones, rnorm2*]
            nc.gpsimd.dma_start(rhsA[:], rhsA_d[:])
            lhsB = cp.tile([5, S * N], F32)   # [-2 rT, rnorm2*, ones]
            nc.gpsimd.dma_start(lhsB[:], lhsB_d[:])
            rhsB = cp.tile([5, S * N], F32)   # [pT, ones, (pnorm2+mbig)*]
            nc.gpsimd.dma_start(rhsB[:], rhsB_d[:])
            rhsC = cp.tile([5, S * N], F32)   # [pT, ones, pnorm2*]      (cd)
            nc.gpsimd.dma_start(rhsC[:], rhsC_d[:])
            lhsK = cp.tile([5, S * K], F32)   # [-2 kT, knorm2*, ones]
            nc.gpsimd.dma_start(lhsK[:], lhsK_d[:])
            rhsD = cp.tile([5, S * K], F32)   # [kT, ones, knorm2*]      (div)
            nc.gpsimd.dma_start(rhsD[:], rhsD_d[:])
            mcols = cp.tile([128, S * T], F32)
            nc.gpsimd.dma_start(mcols[:], mcol[:])
            deltas = cp.tile([128, S * 24], F32)
            nc.gpsimd.dma_start(deltas[:], delta2[:])
            nocsPt = cp.tile([K, S * 3], F32)
            nc.gpsimd.dma_start(nocsPt[:], nocsP[:])
            smallAt = cp.tile([4, 24], F32)
            nc.gpsimd.dma_start(smallAt[:], smallA[:])
            smallBt = cp.tile([S, 1036], F32)
            nc.gpsimd.dma_start(smallBt[:], smallB[:])
            eyet = cp.tile([K, K], F32)
            nc.gpsimd.dma_start(eyet[:], eye96[:])
            constst = cp.tile([128, 97], F32)
            nc.gpsimd.dma_start(constst[:], consts[:])

            # ACT checkpoints for DMA-only tiles that PE will consume
            nc.scalar.activation(mcols[:], mcols[:], AF.Copy)
            nc.scalar.activation(smallAt[:], smallAt[:], AF.Copy)
            nc.scalar.activation(constst[:], constst[:], AF.Copy)

            ones3 = smallAt[0:3, 23:24]
            ones4 = smallAt[0:4, 23:24]
            epsap = smallAt[0:1, 22:23]       # 1e-8
            onescol = constst[:, 0:1]
            ones96 = constst[0:1, 1:97]

            partials = cp.tile([128, W], F32)
            nc.vector.memset(partials[:], 0.0)
            eps96 = cp.tile([K, 1], F32)
            nc.vector.memset(eps96[:], 1e-12)
            negth96 = cp.tile([K, 1], F32)
            nc.vector.memset(negth96[:], -TH)

            # ---- squared coords; psq gets a 4th row = mbig for masked norms
            psq4 = cp.tile([4, S * N], F32)
            nc.scalar.activation(psq4[0:3, :], rhsB[0:3, :], AF.Square)
            nc.gpsimd.dma_start(psq4[3:4, :], mbig_d[:])
            nc.scalar.activation(psq4[:], psq4[:], AF.Copy)   # checkpoint
            rsq = cp.tile([3, S * N], F32)
            nc.scalar.activation(rsq[:], rhsA[0:3, :], AF.Square)
            ksq = cp.tile([3, S * K], F32)
            nc.scalar.activation(ksq[:], rhsD[0:3, :], AF.Square)

            # norm rows staged in SBUF (ACT writers only), then DMA'd into
            # the odd-partition rows of the operand tiles
            pnw = cp.tile([1, S * N], F32)    # pnorm2
            pbigw = cp.tile([1, S * N], F32)  # pnorm2 + mbig
            rnw = cp.tile([1, S * N], F32)    # rnorm2

            for s in range(S):
                for h in range(2):
                    o = s * N + h * 512
                    pm = psaux.tile([1, 512], F32, tag="aux")
                    nc.tensor.matmul(
                        pm[:], ones4, psq4[:, o : o + 512],
                        start=True, stop=True,
                    )
                    nc.scalar.activation(pbigw[:, o : o + 512], pm[:], AF.Copy)
                    pp = psaux.tile([1, 512], F32, tag="aux")
                    nc.tensor.matmul(
                        pp[:], ones3, psq4[0:3, o : o + 512],
                        start=True, stop=True,
                    )
                    nc.scalar.activation(pnw[:, o : o + 512], pp[:], AF.Copy)
                    rn = psaux.tile([1, 512], F32, tag="aux")
                    nc.tensor.matmul(
                        rn[:], ones3, rsq[:, o : o + 512],
                        start=True, stop=True,
                    )
                    nc.scalar.activation(rnw[:, o : o + 512], rn[:], AF.Copy)

            nc.gpsimd.dma_start(rhsC[4:5, :], pnw[:])
            nc.gpsimd.dma_start(rhsB[4:5, :], pbigw[:])
            nc.gpsimd.dma_start(lhsA[3:4, :], pbigw[:])
            nc.gpsimd.dma_start(rhsA[4:5, :], rnw[:])
            nc.gpsimd.dma_start(lhsB[3:4, :], rnw[:])

            kn = psaux.tile([1, S * K], F32, tag="aux")
            nc.tensor.matmul(kn[:], ones3, ksq[:], start=True, stop=True)
            knw = wp.tile([1, S * K], F32, tag="knw")
            nc.scalar.activation(knw[:], kn[:], AF.Copy)
            nc.gpsimd.dma_start(lhsK[3:4, :], knw[:])
            nc.gpsimd.dma_start(rhsD[4:5, :], knw[:])

            # checkpoint: in-place ACT copies make each operand tile's last
            # writer the ACT engine (single PE wait; HW limit is one)
            nc.scalar.activation(lhsA[:], lhsA[:], AF.Copy)
            nc.scalar.activation(rhsA[:], rhsA[:], AF.Copy)
            nc.scalar.activation(lhsB[:], lhsB[:], AF.Copy)
            nc.scalar.activation(rhsB[:], rhsB[:], AF.Copy)
            nc.scalar.activation(rhsC[:], rhsC[:], AF.Copy)
            nc.scalar.activation(lhsK[:], lhsK[:], AF.Copy)
            nc.scalar.activation(rhsD[:], rhsD[:], AF.Copy)

            # ---- size-norm reciprocal, on ACT (partition 0, free dim = s)
            szsqT = wp.tile([3, S], F32, tag="sm")
            nc.scalar.activation(szsqT[:], smallAt[0:3, 20:22], AF.Square)
            psS = psaux.tile([1, S], F32, tag="aux")
            nc.tensor.matmul(psS[:], ones3, szsqT[:], start=True, stop=True)
            snorms = wp.tile([1, S], F32, tag="sm2")
            nc.scalar.activation(snorms[:], psS[:], AF.Sqrt)
            snorme = wp.tile([1, S], F32, tag="sm3")
            nc.vector.tensor_scalar(snorme[:], snorms[:], 1e-8, None, op0=ALU.add)
            recipr = wp.tile([1, S], F32, tag="sm4")
            nc.vector.reciprocal(recipr[:], snorme[:])
            nc.scalar.activation(recipr[:], recipr[:], AF.Copy)  # ACT checkpoint

            # ---- main chamfer / cd loops ----
            scols = cp.tile([128, S * T], F32)

            for s in range(S):
                c0 = s * N
                for t in range(T):
                    lcol = c0 + t * 128
                    # orientation A: pts rows (self-bias = pnorm2 + mbig),
                    # free dim = recon -> row-min = d1 (masked rows ~1e10)
                    psA = psbig.tile([128, N], F32, tag="big")
                    for h in range(2):
                        nc.tensor.matmul(
                            psA[:, h * 512 : (h + 1) * 512],
                            lhsA[:, lcol : lcol + 128],
                            rhsA[:, c0 + h * 512 : c0 + (h + 1) * 512],
                            start=True, stop=True,
                        )
                    dcol = wp.tile([128, 1], F32, tag="dcol")
                    nc.vector.tensor_reduce(dcol[:], psA[:], axis=AX.X, op=ALU.min)
                    dcolc = wp.tile([128, 1], F32, tag="dcolc")
                    nc.vector.tensor_scalar(dcolc[:], dcol[:], 0.0, None, op0=ALU.max)
                    idx = s * T + t
                    nc.scalar.activation(
                        scols[:, idx : idx + 1], dcolc[:], AF.Sqrt
                    )

                    # orientation B: recon rows, free dim = pts (mask folded
                    # into rhsB self-bias) -> row-min = d2
                    psB = psbig.tile([128, N], F32, tag="big")
                    for h in range(2):
                        nc.tensor.matmul(
                            psB[:, h * 512 : (h + 1) * 512],
                            lhsB[:, lcol : lcol + 128],
                            rhsB[:, c0 + h * 512 : c0 + (h + 1) * 512],
                            start=True, stop=True,
                        )
                    dcol2 = wp.tile([128, 1], F32, tag="dcol2")
                    nc.vector.tensor_reduce(dcol2[:], psB[:], axis=AX.X, op=ALU.min)
                    dcol2c = wp.tile([128, 1], F32, tag="dcol2c")
                    nc.vector.tensor_scalar(dcol2c[:], dcol2[:], 0.0, None, op0=ALU.max)
                    sl = SL_D2 + s * T + t
                    nc.scalar.activation(
                        partials[:, sl : sl + 1], dcol2c[:], AF.Sqrt
                    )

                # cd: kpt rows, free dim = pts -> row-min over n1
                psC = psbig.tile([128, N], F32, tag="big")
                for h in range(2):
                    nc.tensor.matmul(
                        psC[0:K, h * 512 : (h + 1) * 512],
                        lhsK[:, s * K : (s + 1) * K],
                        rhsC[:, c0 + h * 512 : c0 + (h + 1) * 512],
                        start=True, stop=True,
                    )
                cdc = wp.tile([K, 1], F32, tag="cdc")
                nc.vector.tensor_reduce(cdc[:], psC[0:K, :], axis=AX.X, op=ALU.min)
                cdcc = wp.tile([K, 1], F32, tag="cdcc")
                nc.vector.tensor_scalar(cdcc[:], cdc[:], 0.0, None, op0=ALU.max)
                nc.scalar.activation(
                    partials[0:K, SL_CD + s : SL_CD + s + 1], cdcc[:], AF.Sqrt
                )

                # diversity: kpt x kpt
                psD = psaux.tile([K, K], F32, tag="aux")
                nc.tensor.matmul(
                    psD[:], lhsK[:, s * K : (s + 1) * K],
                    rhsD[:, s * K : (s + 1) * K], start=True, stop=True,
                )
                dmx = wp.tile([K, K], F32, tag="dmx")
                nc.scalar.activation(dmx[:], psD[:], AF.Relu)
                dm = wp.tile([K, K], F32, tag="dm")
                nc.scalar.activation(dm[:], dmx[:], AF.Sqrt, bias=eps96[:])
                nc.vector.tensor_add(dm[:], dm[:], eyet[:])
                nc.vector.tensor_scalar(dm[:], dm[:], TH, None, op0=ALU.min)
                nc.vector.tensor_reduce(
                    partials[0:K, SL_DIV + s : SL_DIV + s + 1], dm[:],
                    axis=AX.X, op=ALU.add,
                )

                # NOCS smooth-L1; host supplies -t in smallA cols 12:14
                amt = wp.tile([3, K], F32, tag="amt")
                nc.scalar.activation(
                    amt[:], rhsD[0:3, s * K : (s + 1) * K], AF.Identity,
                    bias=smallAt[0:3, 12 + s : 13 + s],
                )
                psN = psaux.tile([K, 3], F32, tag="aux")
                nc.tensor.matmul(
                    psN[:], amt[:], smallAt[0:3, 14 + 3 * s : 17 + 3 * s],
                    start=True, stop=True,
                )
                psR = psaux.tile([K, 1], F32, tag="aux")
                nc.tensor.matmul(
                    psR[:], ones96, recipr[0:1, s : s + 1],
                    start=True, stop=True,
                )
                recip96 = wp.tile([K, 1], F32, tag="r96")
                nc.scalar.activation(recip96[:], psR[:], AF.Copy)
                gd = wp.tile([K, 3], F32, tag="gd")
                nc.vector.scalar_tensor_tensor(
                    gd[:], psN[:], recip96[:], nocsPt[:, 3 * s : 3 * s + 3],
                    op0=ALU.mult, op1=ALU.subtract,
                )
                ad = wp.tile([K, 3], F32, tag="ad")
                nc.scalar.activation(ad[:], gd[:], AF.Abs)
                uu = wp.tile([K, 3], F32, tag="uu")
                nc.vector.tensor_scalar(uu[:], ad[:], TH, None, op0=ALU.min)
                u5 = wp.tile([K, 3], F32, tag="u5")
                nc.vector.scalar_tensor_tensor(
                    u5[:], uu[:], 1.0 / (2.0 * TH), uu[:],
                    op0=ALU.mult, op1=ALU.mult,
                )
                vv = wp.tile([K, 3], F32, tag="vv")
                nc.scalar.activation(vv[:], ad[:], AF.Relu, bias=negth96[:])
                sl1 = wp.tile([K, 3], F32, tag="sl1")
                nc.vector.tensor_add(sl1[:], u5[:], vv[:])
                nc.vector.tensor_reduce(
                    partials[0:K, SL_NOCS + s : SL_NOCS + s + 1], sl1[:],
                    axis=AX.X, op=ALU.add,
                )

                # rotation column norms (inputs pre-transposed on host)
                rd = wp.tile([3, 3], F32, tag="rd")
                nc.vector.tensor_sub(
                    rd[:], smallAt[0:3, 6 * s : 6 * s + 3],
                    smallAt[0:3, 6 * s + 3 : 6 * s + 6],
                )
                rdsq = wp.tile([3, 3], F32, tag="rdsq")
                nc.vector.tensor_mul(rdsq[:], rd[:], rd[:])
                rn3 = wp.tile([3, 1], F32, tag="rn3")
                nc.vector.tensor_reduce(rn3[:], rdsq[:], axis=AX.X, op=ALU.add)
                nc.scalar.activation(
                    partials[0:3, SL_ROT + s : SL_ROT + s + 1], rn3[:], AF.Sqrt
                )

            # ---- recon_delta norms ----
            dsq = wp.tile([128, S * 24], F32, tag="dsq")
            nc.vector.tensor_mul(dsq[:], deltas[:], deltas[:])
            dn = wp.tile([128, S * 8], F32, tag="dn")
            nc.vector.tensor_reduce(
                dn[:], dsq[:].rearrange("p (x c) -> p x c", c=3),
                axis=AX.X, op=ALU.add,
            )
            dns = wp.tile([128, S * 8], F32, tag="dns")
            nc.scalar.activation(dns[:], dn[:], AF.Sqrt)
            for s in range(S):
                nc.vector.tensor_reduce(
                    partials[:, SL_DELTA + s : SL_DELTA + s + 1],
                    dns[:, s * 8 : (s + 1) * 8], axis=AX.X, op=ALU.add,
                )

            # ---- translation / size / distillation norms (rows = samples) ----
            for slot, a0, b0, width in (
                (SL_TRANS, 0, 3, 3),
                (SL_SIZE, 6, 9, 3),
                (SL_DIST, 12, 524, 512),
            ):
                df = wp.tile([S, width], F32, tag=f"df{slot}")
                nc.vector.tensor_sub(
                    df[:], smallBt[:, a0 : a0 + width],
                    smallBt[:, b0 : b0 + width],
                )
                dfsq = wp.tile([S, width], F32, tag=f"dfsq{slot}")
                nc.vector.tensor_mul(dfsq[:], df[:], df[:])
                dfs = wp.tile([S, 1], F32, tag=f"dfs{slot}")
                nc.vector.tensor_reduce(dfs[:], dfsq[:], axis=AX.X, op=ALU.add)
                nc.scalar.activation(
                    partials[0:S, slot : slot + 1], dfs[:], AF.Sqrt
                )

            # ---- masked d1 sums: diag(mcols^T @ scols) ----
            psD1 = psfin.tile([S * T, S * T], F32, tag="fin")
            nc.tensor.matmul(psD1[:], mcols[:], scols[:], start=True, stop=True)
            outs2 = wp.tile([S * T, S * T], F32, tag="outs2")
            nc.scalar.activation(outs2[:], psD1[:], AF.Copy)
            nc.gpsimd.dma_start(out2[:], outs2[:])

            # ---- collapse partition dim: out[w] = sum_p partials[p, w] ----
            nc.scalar.activation(partials[:], partials[:], AF.Copy)  # checkpoint
            psF = psfin.tile([W, 1], F32, tag="fin")
            nc.tensor.matmul(psF[:], partials[:], onescol, start=True, stop=True)
            outs = wp.tile([W, 1], F32, tag="outs")
            nc.scalar.activation(outs[:], psF[:], AF.Copy)
            nc.gpsimd.dma_start(out[:], outs[:])

    nc.finalize()
    return nc


def _get_program():
    global _PROGRAM
    if _PROGRAM is None:
        _PROGRAM = _build_program()
    return _PROGRAM


def _make_in_maps(pts, recon_delta, pred_kpt_3d, recon_model, translation_label,
                  rotation_label, size_label, pred_rotation, pred_translation,
                  pred_size, pred_kpt_nocs, pc_mask, distillation_pointnet,
                  distillation_ulip):
    f32 = np.float32
    eye = np.ascontiguousarray(np.eye(K, dtype=f32))
    consts = np.zeros((128, 97), f32)
    consts[:, 0] = 1.0
    consts[0, 1:97] = 1.0
    in_maps = []
    for c in range(NCORES):
        sl = slice(S * c, S * c + S)
        p = np.asarray(pts[sl], f32)
        r = np.asarray(recon_model[sl], f32)
        kp = np.asarray(pred_kpt_3d[sl], f32)
        m = np.asarray(pc_mask[sl]).astype(f32)

        ptsT = p.transpose(2, 0, 1).reshape(3, S * N)
        reconT = r.transpose(2, 0, 1).reshape(3, S * N)
        kptT = kp.transpose(2, 0, 1).reshape(3, S * K)
        mbig = np.ascontiguousarray(((1.0 - m) * BIGSQ).reshape(1, S * N))

        lhsA = np.ones((5, S * N), f32)
        lhsA[0:3] = -2.0 * ptsT
        rhsA = np.ones((5, S * N), f32)
        rhsA[0:3] = reconT
        lhsB = np.ones((5, S * N), f32)
        lhsB[0:3] = -2.0 * reconT
        rhsB = np.ones((5, S * N), f32)
        rhsB[0:3] = ptsT
        rhsC = np.ones((5, S * N), f32)
        rhsC[0:3] = ptsT
        lhsK = np.ones((5, S * K), f32)
        lhsK[0:3] = -2.0 * kptT
        rhsD = np.ones((5, S * K), f32)
        rhsD[0:3] = kptT

        mcolh = np.ascontiguousarray(
            m.reshape(S, T, 128).transpose(2, 0, 1).reshape(128, S * T))
        delta2 = np.ascontiguousarray(
            np.asarray(recon_delta[sl], f32)
            .reshape(S, 128, 8, 3).transpose(1, 0, 2, 3).reshape(128, S * 24))
        nocsPh = np.ascontiguousarray(
            np.asarray(pred_kpt_nocs[sl], f32).transpose(1, 0, 2).reshape(K, S * 3))

        Rp = np.asarray(pred_rotation[sl], f32)
        Rl = np.asarray(rotation_label[sl], f32)
        smallA = np.zeros((4, 24), f32)
        smallA[0:3, 0:3] = Rp[0].T
        smallA[0:3, 3:6] = Rl[0].T
        smallA[0:3, 6:9] = Rp[1].T
        smallA[0:3, 9:12] = Rl[1].T
        smallA[0:3, 12:14] = -np.asarray(translation_label[sl], f32).T
        smallA[0:3, 14:17] = Rl[0]
        smallA[0:3, 17:20] = Rl[1]
        smallA[0:3, 20:22] = np.asarray(size_label[sl], f32).T
        smallA[0, 22] = 1e-8
        smallA[0:4, 23] = 1.0

        smallB = np.zeros((S, 1036), f32)
        smallB[:, 0:3] = np.asarray(pred_translation[sl], f32)
        smallB[:, 3:6] = np.asarray(translation_label[sl], f32)
        smallB[:, 6:9] = np.asarray(pred_size[sl], f32)
        smallB[:, 9:12] = np.asarray(size_label[sl], f32)
        smallB[:, 12:524] = np.asarray(distillation_pointnet[sl], f32)
        smallB[:, 524:1036] = np.asarray(distillation_ulip[sl], f32)

        in_maps.append(dict(
            lhsA=lhsA, rhsA=rhsA, lhsB=lhsB, rhsB=rhsB, rhsC=rhsC,
            lhsK=lhsK, rhsD=rhsD, mcol=mcolh, delta2=delta2, nocsP=nocsPh,
            smallA=smallA, smallB=smallB, eye96=eye, mbig=mbig,
            consts=consts,
        ))
    return in_maps


def _combine(results, pc_mask):
    msum = np.asarray(pc_mask).astype(np.float64).sum(axis=1)  # (16,)
    d1m = np.zeros(B)
    d2 = np.zeros(B)
    cd = div = nocs = rot = delta = trans = size = dist = 0.0
    for c in range(NCORES):
        parts = np.asarray(results[c]["out"], np.float64).reshape(W)
        parts2 = np.diag(np.asarray(results[c]["out2"], np.float64)
                         .reshape(S * T, S * T))
        for s in range(S):
            g = S * c + s
            d1m[g] = parts2[s * T : (s + 1) * T].sum() / msum[g]
            d2[g] = parts[SL_D2 + s * T : SL_D2 + (s + 1) * T].sum() / N
            cd += parts[SL_CD + s]
            div += (K * K - parts[SL_DIV + s] / TH) / (K * (K - 1))
            nocs += parts[SL_NOCS + s]
            rot += parts[SL_ROT + s]
        delta += parts[SL_DELTA] + parts[SL_DELTA + 1]
        trans += parts[SL_TRANS]
        size += parts[SL_SIZE]
        dist += parts[SL_DIST]

    loss_pose = rot / (B * 3) + trans / B + size / B
    loss_nocs = nocs / (B * K)
    loss_cd = cd / (B * K)
    loss_div = div / B
    loss_recon = float(np.mean(0.5 * d1m + 0.5 * d2))
    loss_delta = delta / (B * N)
    loss_dist = dist / B
    terms = np.array([loss_pose, loss_nocs, loss_cd, loss_div, loss_recon,
                      loss_delta, loss_dist], np.float64)
    return np.concatenate([[terms.sum()], terms]).astype(np.float32)


def kernel(**inputs):
    nc = _get_program()
    in_maps = _make_in_maps(**inputs)
    res = run_bass_kernel_spmd(nc, in_maps, list(range(NCORES)))
    return _combine(res.results, inputs["pc_mask"])


# revision 43
# speedup vs baseline: 1.0297x; 1.0297x over previous
"""Trainium2 Bass kernel for nn_Loss_50989851738757 (multi-term pose/chamfer loss).

Strategy: pure data parallel over batch (b=16 -> 2 samples per core, 8 cores).
Each core computes, for its 2 samples, per-sample partial sums of every loss
term; a ones-matmul collapses the partition dim and ~64 partial scalars per
core are shipped back.  Host does the final (tiny) means + weighted sum.

Key device-side tricks:
- every pairwise-distance matrix is produced directly in PSUM via an
  augmented K=5 matmul
    lhsT = [-2*a_x, -2*a_y, -2*a_z, selfbias_a, ones]
    rhs  = [ b_x,    b_y,    b_z,   ones,       selfbias_b]
  so psum[i,j] = ||a_i||^2 + ||b_j||^2 - 2 a_i.b_j  (+ mask*1e20 folded into
  the pts selfbias for the chamfer orientations that need masking).
- min(sqrt(x)) == sqrt(min(x)): only min-reduced columns get sqrt'ed.
- the masked d1 sums come from one cross-matmul mcols^T @ scols whose
  diagonal is the per-row-tile masked sum.
- built on Bacc so generate_event_semaphores legalizes the 1-wait-per-
  instruction HW constraint automatically.
"""

import sys

import numpy as np

sys.path.insert(0, "/opt/trn_rl_repo")

import concourse.bass as bass
import concourse.bacc as bacc
import concourse.mybir as mybir
from concourse.bass_utils import run_bass_kernel_spmd
from concourse.tile import TileContext

F32 = mybir.dt.float32
AF = mybir.ActivationFunctionType
ALU = mybir.AluOpType
AX = mybir.AxisListType

NCORES = 8
B = 16
S = 2          # samples per core
N = 1024       # n1 == n2
K = 96         # keypoints
T = 8          # 128-row tiles per sample
BIGSQ = 1e20   # mask fill, pre-sqrt (sqrt -> 1e10 like the reference BIG)
TH = 0.1
W = 48         # partial slots

# partial slot map (columns of the (128, W) partials tile)
SL_D2 = 16     # 16..31  d2 sums            (s*8 + t)
SL_CD = 32     # 32, 33  cd min-dist sums   (rows 0:96)
SL_DIV = 34    # 34, 35  sum of clipped diversity dist (rows 0:96)
SL_NOCS = 36   # 36, 37  smooth-l1 sums     (rows 0:96)
SL_ROT = 38    # 38, 39  rotation col-norm sums (rows 0:3)
SL_DELTA = 40  # 40, 41  delta norm sums    (rows 0:128)
SL_TRANS = 42  # translation norms, rows 0:2 (one per sample)
SL_SIZE = 43   # size norms, rows 0:2
SL_DIST = 44   # distillation norms, rows 0:2

_PROGRAM = None


def _build_program():
    nc = bacc.Bacc()

    lhsA_d = nc.dram_tensor("lhsA", [5, S * N], F32, kind="ExternalInput")
    rhsA_d = nc.dram_tensor("rhsA", [5, S * N], F32, kind="ExternalInput")
    lhsB_d = nc.dram_tensor("lhsB", [5, S * N], F32, kind="ExternalInput")
    rhsB_d = nc.dram_tensor("rhsB", [5, S * N], F32, kind="ExternalInput")
    rhsC_d = nc.dram_tensor("rhsC", [5, S * N], F32, kind="ExternalInput")
    lhsK_d = nc.dram_tensor("lhsK", [5, S * K], F32, kind="ExternalInput")
    rhsD_d = nc.dram_tensor("rhsD", [5, S * K], F32, kind="ExternalInput")
    mcol = nc.dram_tensor("mcol", [128, S * T], F32, kind="ExternalInput")
    delta2 = nc.dram_tensor("delta2", [128, S * 24], F32, kind="ExternalInput")
    nocsP = nc.dram_tensor("nocsP", [K, S * 3], F32, kind="ExternalInput")
    smallA = nc.dram_tensor("smallA", [4, 24], F32, kind="ExternalInput")
    smallB = nc.dram_tensor("smallB", [S, 1036], F32, kind="ExternalInput")
    eye96 = nc.dram_tensor("eye96", [K, K], F32, kind="ExternalInput")
    mbig_d = nc.dram_tensor("mbig", [1, S * N], F32, kind="ExternalInput")
    consts = nc.dram_tensor("consts", [128, 97], F32, kind="ExternalInput")
    out = nc.dram_tensor("out", [W, 1], F32, kind="ExternalOutput")
    out2 = nc.dram_tensor("out2", [S * T, S * T], F32, kind="ExternalOutput")

    with TileContext(nc) as tc:
        with (
            tc.tile_pool(name="const", bufs=1) as cp,
            tc.tile_pool(name="work", bufs=3) as wp,
            tc.tile_pool(name="psbig", bufs=2, space="PSUM") as psbig,
            tc.tile_pool(name="psaux", bufs=3, space="PSUM") as psaux,
            tc.tile_pool(name="psfin", bufs=1, space="PSUM") as psfin,
        ):
            # ---- load inputs (one DMA per operand tile) ----
            lhsA = cp.tile([5, S * N], F32)   # [-2 pT, (pnorm2+mbig)*, ones]
            nc.gpsimd.dma_start(lhsA[:], lhsA_d[:])
            rhsA = cp.tile([5, S * N], F32)   # [rT, ones, rnorm2*]
            nc.gpsimd.dma_start(rhsA[:], rhsA_d[:])
            lhsB = cp.tile([5, S * N], F32)   # [-2 rT, rnorm2*, ones]
            nc.gpsimd.dma_start(lhsB[:], lhsB_d[:])
            rhsB = cp.tile([5, S * N], F32)   # [pT, ones, (pnorm2+mbig)*]
            nc.gpsimd.dma_start(rhsB[:], rhsB_d[:])
            rhsC = cp.tile([5, S * N], F32)   # [pT, ones, pnorm2*]      (cd)
            nc.gpsimd.dma_start(rhsC[:], rhsC_d[:])
            lhsK = cp.tile([5, S * K], F32)   # [-2 kT, knorm2*, ones]
            nc.gpsimd.dma_start(lhsK[:], lhsK_d[:])
            rhsD = cp.tile([5, S * K], F32)   # [kT, ones, knorm2*]      (div)
            nc.gpsimd.dma_start(rhsD[:], rhsD_d[:])
            mcols = cp.tile([128, S * T], F32)
            nc.gpsimd.dma_start(mcols[:], mcol[:])
            deltas = cp.tile([128, S * 24], F32)
            nc.gpsimd.dma_start(deltas[:], delta2[:])
            nocsPt = cp.tile([K, S * 3], F32)
            nc.gpsimd.dma_start(nocsPt[:], nocsP[:])
            smallAt = cp.tile([4, 24], F32)
            nc.gpsimd.dma_start(smallAt[:], smallA[:])
            smallBt = cp.tile([S, 1036], F32)
            nc.gpsimd.dma_start(smallBt[:], smallB[:])
            eyet = cp.tile([K, K], F32)
            nc.gpsimd.dma_start(eyet[:], eye96[:])
            constst = cp.tile([128, 97], F32)
            nc.gpsimd.dma_start(constst[:], consts[:])


            ones3 = smallAt[0:3, 23:24]
            ones4 = smallAt[0:4, 23:24]
            epsap = smallAt[0:1, 22:23]       # 1e-8
            onescol = constst[:, 0:1]
            ones96 = constst[0:1, 1:97]

            partials = cp.tile([128, W], F32)
            nc.vector.memset(partials[:], 0.0)
            eps96 = cp.tile([K, 1], F32)
            nc.vector.memset(eps96[:], 1e-12)
            negth96 = cp.tile([K, 1], F32)
            nc.vector.memset(negth96[:], -TH)

            # ---- squared coords; psq gets a 4th row = mbig for masked norms
            psq4 = cp.tile([4, S * N], F32)
            nc.scalar.activation(psq4[0:3, :], rhsB[0:3, :], AF.Square)
            nc.gpsimd.dma_start(psq4[3:4, :], mbig_d[:])
            rsq = cp.tile([3, S * N], F32)
            nc.scalar.activation(rsq[:], rhsA[0:3, :], AF.Square)
            ksq = cp.tile([3, S * K], F32)
            nc.scalar.activation(ksq[:], rhsD[0:3, :], AF.Square)

            # norm rows staged in SBUF (ACT writers only), then DMA'd into
            # the odd-partition rows of the operand tiles
            pnw = cp.tile([1, S * N], F32)    # pnorm2
            pbigw = cp.tile([1, S * N], F32)  # pnorm2 + mbig
            rnw = cp.tile([1, S * N], F32)    # rnorm2

            for s in range(S):
                for h in range(2):
                    o = s * N + h * 512
                    pm = psaux.tile([1, 512], F32, tag="aux")
                    nc.tensor.matmul(
                        pm[:], ones4, psq4[:, o : o + 512],
                        start=True, stop=True,
                    )
                    nc.scalar.activation(pbigw[:, o : o + 512], pm[:], AF.Copy)
                    pp = psaux.tile([1, 512], F32, tag="aux")
                    nc.tensor.matmul(
                        pp[:], ones3, psq4[0:3, o : o + 512],
                        start=True, stop=True,
                    )
                    nc.scalar.activation(pnw[:, o : o + 512], pp[:], AF.Copy)
                    rn = psaux.tile([1, 512], F32, tag="aux")
                    nc.tensor.matmul(
                        rn[:], ones3, rsq[:, o : o + 512],
                        start=True, stop=True,
                    )
                    nc.scalar.activation(rnw[:, o : o + 512], rn[:], AF.Copy)

            nc.gpsimd.dma_start(rhsC[4:5, :], pnw[:])
            nc.gpsimd.dma_start(rhsB[4:5, :], pbigw[:])
            nc.gpsimd.dma_start(lhsA[3:4, :], pbigw[:])
            nc.gpsimd.dma_start(rhsA[4:5, :], rnw[:])
            nc.gpsimd.dma_start(lhsB[3:4, :], rnw[:])

            kn = psaux.tile([1, S * K], F32, tag="aux")
            nc.tensor.matmul(kn[:], ones3, ksq[:], start=True, stop=True)
            knw = wp.tile([1, S * K], F32, tag="knw")
            nc.scalar.activation(knw[:], kn[:], AF.Copy)
            nc.gpsimd.dma_start(lhsK[3:4, :], knw[:])
            nc.gpsimd.dma_start(rhsD[4:5, :], knw[:])


            # ---- size-norm reciprocal, on ACT (partition 0, free dim = s)
            szsqT = wp.tile([3, S], F32, tag="sm")
            nc.scalar.activation(szsqT[:], smallAt[0:3, 20:22], AF.Square)
            psS = psaux.tile([1, S], F32, tag="aux")
            nc.tensor.matmul(psS[:], ones3, szsqT[:], start=True, stop=True)
            snorms = wp.tile([1, S], F32, tag="sm2")
            nc.scalar.activation(snorms[:], psS[:], AF.Sqrt)
            snorme = wp.tile([1, S], F32, tag="sm3")
            nc.vector.tensor_scalar(snorme[:], snorms[:], 1e-8, None, op0=ALU.add)
            recipr = wp.tile([1, S], F32, tag="sm4")
            nc.vector.reciprocal(recipr[:], snorme[:])

            # ---- main chamfer / cd loops ----
            scols = cp.tile([128, S * T], F32)

            for s in range(S):
                c0 = s * N
                for t in range(T):
                    lcol = c0 + t * 128
                    # orientation A: pts rows (self-bias = pnorm2 + mbig),
                    # free dim = recon -> row-min = d1 (masked rows ~1e10)
                    psA = psbig.tile([128, N], F32, tag="big")
                    for h in range(2):
                        nc.tensor.matmul(
                            psA[:, h * 512 : (h + 1) * 512],
                            lhsA[:, lcol : lcol + 128],
                            rhsA[:, c0 + h * 512 : c0 + (h + 1) * 512],
                            start=True, stop=True,
                        )
                    dcol = wp.tile([128, 1], F32, tag="dcol")
                    nc.vector.tensor_reduce(dcol[:], psA[:], axis=AX.X, op=ALU.min)
                    dcolc = wp.tile([128, 1], F32, tag="dcolc")
                    nc.vector.tensor_scalar(dcolc[:], dcol[:], 0.0, None, op0=ALU.max)
                    idx = s * T + t
                    nc.scalar.activation(
                        scols[:, idx : idx + 1], dcolc[:], AF.Sqrt
                    )

                    # orientation B: recon rows, free dim = pts (mask folded
                    # into rhsB self-bias) -> row-min = d2
                    psB = psbig.tile([128, N], F32, tag="big")
                    for h in range(2):
                        nc.tensor.matmul(
                            psB[:, h * 512 : (h + 1) * 512],
                            lhsB[:, lcol : lcol + 128],
                            rhsB[:, c0 + h * 512 : c0 + (h + 1) * 512],
                            start=True, stop=True,
                        )
                    dcol2 = wp.tile([128, 1], F32, tag="dcol2")
                    nc.vector.tensor_reduce(dcol2[:], psB[:], axis=AX.X, op=ALU.min)
                    dcol2c = wp.tile([128, 1], F32, tag="dcol2c")
                    nc.vector.tensor_scalar(dcol2c[:], dcol2[:], 0.0, None, op0=ALU.max)
                    sl = SL_D2 + s * T + t
                    nc.scalar.activation(
                        partials[:, sl : sl + 1], dcol2c[:], AF.Sqrt
                    )

                # cd: kpt rows, free dim = pts -> row-min over n1
                psC = psbig.tile([128, N], F32, tag="big")
                for h in range(2):
                    nc.tensor.matmul(
                        psC[0:K, h * 512 : (h + 1) * 512],
                        lhsK[:, s * K : (s + 1) * K],
                        rhsC[:, c0 + h * 512 : c0 + (h + 1) * 512],
                        start=True, stop=True,
                    )
                cdc = wp.tile([K, 1], F32, tag="cdc")
                nc.vector.tensor_reduce(cdc[:], psC[0:K, :], axis=AX.X, op=ALU.min)
                cdcc = wp.tile([K, 1], F32, tag="cdcc")
                nc.vector.tensor_scalar(cdcc[:], cdc[:], 0.0, None, op0=ALU.max)
                nc.scalar.activation(
                    partials[0:K, SL_CD + s : SL_CD + s + 1], cdcc[:], AF.Sqrt
                )

                # diversity: kpt x kpt
                psD = psaux.tile([K, K], F32, tag="aux")
                nc.tensor.matmul(
                    psD[:], lhsK[:, s * K : (s + 1) * K],
                    rhsD[:, s * K : (s + 1) * K], start=True, stop=True,
                )
                dmx = wp.tile([K, K], F32, tag="dmx")
                nc.scalar.activation(dmx[:], psD[:], AF.Relu)
                dm = wp.tile([K, K], F32, tag="dm")
                nc.scalar.activation(dm[:], dmx[:], AF.Sqrt, bias=eps96[:])
                nc.vector.tensor_add(dm[:], dm[:], eyet[:])
                nc.vector.tensor_scalar(dm[:], dm[:], TH, None, op0=ALU.min)
                nc.vector.tensor_reduce(
                    partials[0:K, SL_DIV + s : SL_DIV + s + 1], dm[:],
                    axis=AX.X, op=ALU.add,
                )

                # NOCS smooth-L1; host supplies -t in smallA cols 12:14
                amt = wp.tile([3, K], F32, tag="amt")
                nc.scalar.activation(
                    amt[:], rhsD[0:3, s * K : (s + 1) * K], AF.Identity,
                    bias=smallAt[0:3, 12 + s : 13 + s],
                )
                psN = psaux.tile([K, 3], F32, tag="aux")
                nc.tensor.matmul(
                    psN[:], amt[:], smallAt[0:3, 14 + 3 * s : 17 + 3 * s],
                    start=True, stop=True,
                )
                psR = psaux.tile([K, 1], F32, tag="aux")
                nc.tensor.matmul(
                    psR[:], ones96, recipr[0:1, s : s + 1],
                    start=True, stop=True,
                )
                recip96 = wp.tile([K, 1], F32, tag="r96")
                nc.scalar.activation(recip96[:], psR[:], AF.Copy)
                gd = wp.tile([K, 3], F32, tag="gd")
                nc.vector.scalar_tensor_tensor(
                    gd[:], psN[:], recip96[:], nocsPt[:, 3 * s : 3 * s + 3],
                    op0=ALU.mult, op1=ALU.subtract,
                )
                ad = wp.tile([K, 3], F32, tag="ad")
                nc.scalar.activation(ad[:], gd[:], AF.Abs)
                uu = wp.tile([K, 3], F32, tag="uu")
                nc.vector.tensor_scalar(uu[:], ad[:], TH, None, op0=ALU.min)
                u5 = wp.tile([K, 3], F32, tag="u5")
                nc.vector.scalar_tensor_tensor(
                    u5[:], uu[:], 1.0 / (2.0 * TH), uu[:],
                    op0=ALU.mult, op1=ALU.mult,
                )
                vv = wp.tile([K, 3], F32, tag="vv")
                nc.scalar.activation(vv[:], ad[:], AF.Relu, bias=negth96[:])
                sl1 = wp.tile([K, 3], F32, tag="sl1")
                nc.vector.tensor_add(sl1[:], u5[:], vv[:])
                nc.vector.tensor_reduce(
                    partials[0:K, SL_NOCS + s : SL_NOCS + s + 1], sl1[:],
                    axis=AX.X, op=ALU.add,
                )

                # rotation column norms (inputs pre-transposed on host)
                rd = wp.tile([3, 3], F32, tag="rd")
                nc.vector.tensor_sub(
                    rd[:], smallAt[0:3, 6 * s : 6 * s + 3],
                    smallAt[0:3, 6 * s + 3 : 6 * s + 6],
                )
                rdsq = wp.tile([3, 3], F32, tag="rdsq")
                nc.vector.tensor_mul(rdsq[:], rd[:], rd[:])
                rn3 = wp.tile([3, 1], F32, tag="rn3")
                nc.vector.tensor_reduce(rn3[:], rdsq[:], axis=AX.X, op=ALU.add)
                nc.scalar.activation(
                    partials[0:3, SL_ROT + s : SL_ROT + s + 1], rn3[:], AF.Sqrt
                )

            # ---- recon_delta norms ----
            dsq = wp.tile([128, S * 24], F32, tag="dsq")
            nc.vector.tensor_mul(dsq[:], deltas[:], deltas[:])
            dn = wp.tile([128, S * 8], F32, tag="dn")
            nc.vector.tensor_reduce(
                dn[:], dsq[:].rearrange("p (x c) -> p x c", c=3),
                axis=AX.X, op=ALU.add,
            )
            dns = wp.tile([128, S * 8], F32, tag="dns")
            nc.scalar.activation(dns[:], dn[:], AF.Sqrt)
            for s in range(S):
                nc.vector.tensor_reduce(
                    partials[:, SL_DELTA + s : SL_DELTA + s + 1],
                    dns[:, s * 8 : (s + 1) * 8], axis=AX.X, op=ALU.add,
                )

            # ---- translation / size / distillation norms (rows = samples) ----
            for slot, a0, b0, width in (
                (SL_TRANS, 0, 3, 3),
                (SL_SIZE, 6, 9, 3),
                (SL_DIST, 12, 524, 512),
            ):
                df = wp.tile([S, width], F32, tag=f"df{slot}")
                nc.vector.tensor_sub(
                    df[:], smallBt[:, a0 : a0 + width],
                    smallBt[:, b0 : b0 + width],
                )
                dfsq = wp.tile([S, width], F32, tag=f"dfsq{slot}")
                nc.vector.tensor_mul(dfsq[:], df[:], df[:])
                dfs = wp.tile([S, 1], F32, tag=f"dfs{slot}")
                nc.vector.tensor_reduce(dfs[:], dfsq[:], axis=AX.X, op=ALU.add)
                nc.scalar.activation(
                    partials[0:S, slot : slot + 1], dfs[:], AF.Sqrt
                )

            # ---- masked d1 sums: diag(mcols^T @ scols) ----
            psD1 = psfin.tile([S * T, S * T], F32, tag="fin")
            nc.tensor.matmul(psD1[:], mcols[:], scols[:], start=True, stop=True)
            outs2 = wp.tile([S * T, S * T], F32, tag="outs2")
            nc.scalar.activation(outs2[:], psD1[:], AF.Copy)
            nc.gpsimd.dma_start(out2[:], outs2[:])

            # ---- collapse partition dim: out[w] = sum_p partials[p, w] ----
            psF = psfin.tile([W, 1], F32, tag="fin")
            nc.tensor.matmul(psF[:], partials[:], onescol, start=True, stop=True)
            outs = wp.tile([W, 1], F32, tag="outs")
            nc.scalar.activation(outs[:], psF[:], AF.Copy)
            nc.gpsimd.dma_start(out[:], outs[:])

    nc.finalize()
    return nc


def _get_program():
    global _PROGRAM
    if _PROGRAM is None:
        _PROGRAM = _build_program()
    return _PROGRAM


def _make_in_maps(pts, recon_delta, pred_kpt_3d, recon_model, translation_label,
                  rotation_label, size_label, pred_rotation, pred_translation,
                  pred_size, pred_kpt_nocs, pc_mask, distillation_pointnet,
                  distillation_ulip):
    f32 = np.float32
    eye = np.ascontiguousarray(np.eye(K, dtype=f32))
    consts = np.zeros((128, 97), f32)
    consts[:, 0] = 1.0
    consts[0, 1:97] = 1.0
    in_maps = []
    for c in range(NCORES):
        sl = slice(S * c, S * c + S)
        p = np.asarray(pts[sl], f32)
        r = np.asarray(recon_model[sl], f32)
        kp = np.asarray(pred_kpt_3d[sl], f32)
        m = np.asarray(pc_mask[sl]).astype(f32)

        ptsT = p.transpose(2, 0, 1).reshape(3, S * N)
        reconT = r.transpose(2, 0, 1).reshape(3, S * N)
        kptT = kp.transpose(2, 0, 1).reshape(3, S * K)
        mbig = np.ascontiguousarray(((1.0 - m) * BIGSQ).reshape(1, S * N))

        lhsA = np.ones((5, S * N), f32)
        lhsA[0:3] = -2.0 * ptsT
        rhsA = np.ones((5, S * N), f32)
        rhsA[0:3] = reconT
        lhsB = np.ones((5, S * N), f32)
        lhsB[0:3] = -2.0 * reconT
        rhsB = np.ones((5, S * N), f32)
        rhsB[0:3] = ptsT
        rhsC = np.ones((5, S * N), f32)
        rhsC[0:3] = ptsT
        lhsK = np.ones((5, S * K), f32)
        lhsK[0:3] = -2.0 * kptT
        rhsD = np.ones((5, S * K), f32)
        rhsD[0:3] = kptT

        mcolh = np.ascontiguousarray(
            m.reshape(S, T, 128).transpose(2, 0, 1).reshape(128, S * T))
        delta2 = np.ascontiguousarray(
            np.asarray(recon_delta[sl], f32)
            .reshape(S, 128, 8, 3).transpose(1, 0, 2, 3).reshape(128, S * 24))
        nocsPh = np.ascontiguousarray(
            np.asarray(pred_kpt_nocs[sl], f32).transpose(1, 0, 2).reshape(K, S * 3))

        Rp = np.asarray(pred_rotation[sl], f32)
        Rl = np.asarray(rotation_label[sl], f32)
        smallA = np.zeros((4, 24), f32)
        smallA[0:3, 0:3] = Rp[0].T
        smallA[0:3, 3:6] = Rl[0].T
        smallA[0:3, 6:9] = Rp[1].T
        smallA[0:3, 9:12] = Rl[1].T
        smallA[0:3, 12:14] = -np.asarray(translation_label[sl], f32).T
        smallA[0:3, 14:17] = Rl[0]
        smallA[0:3, 17:20] = Rl[1]
        smallA[0:3, 20:22] = np.asarray(size_label[sl], f32).T
        smallA[0, 22] = 1e-8
        smallA[0:4, 23] = 1.0

        smallB = np.zeros((S, 1036), f32)
        smallB[:, 0:3] = np.asarray(pred_translation[sl], f32)
        smallB[:, 3:6] = np.asarray(translation_label[sl], f32)
        smallB[:, 6:9] = np.asarray(pred_size[sl], f32)
        smallB[:, 9:12] = np.asarray(size_label[sl], f32)
        smallB[:, 12:524] = np.asarray(distillation_pointnet[sl], f32)
        smallB[:, 524:1036] = np.asarray(distillation_ulip[sl], f32)

        in_maps.append(dict(
            lhsA=lhsA, rhsA=rhsA, lhsB=lhsB, rhsB=rhsB, rhsC=rhsC,
            lhsK=lhsK, rhsD=rhsD, mcol=mcolh, delta2=delta2, nocsP=nocsPh,
            smallA=smallA, smallB=smallB, eye96=eye, mbig=mbig,
            consts=consts,
        ))
    return in_maps


def _combine(results, pc_mask):
    msum = np.asarray(pc_mask).astype(np.float64).sum(axis=1)  # (16,)
    d1m = np.zeros(B)
    d2 = np.zeros(B)
    cd = div = nocs = rot = delta = trans = size = dist = 0.0
    for c in range(NCORES):
        parts = np.asarray(results[c]["out"], np.float64).reshape(W)
        parts2 = np.diag(np.asarray(results[c]["out2"], np.float64)
                         .reshape(S * T, S * T))
        for s in range(S):
            g = S * c + s
            d1m[g] = parts2[s * T : (s + 1) * T].sum() / msum[g]
            d2[g] = parts[SL_D2 + s * T : SL_D2 + (s + 1) * T].sum() / N
            cd += parts[SL_CD + s]
            div += (K * K - parts[SL_DIV + s] / TH) / (K * (K - 1))
            nocs += parts[SL_NOCS + s]
            rot += parts[SL_ROT + s]
        delta += parts[SL_DELTA] + parts[SL_DELTA + 1]
        trans += parts[SL_TRANS]
        size += parts[SL_SIZE]
        dist += parts[SL_DIST]

    loss_pose = rot / (B * 3) + trans / B + size / B
    loss_nocs = nocs / (B * K)
    loss_cd = cd / (B * K)
    loss_div = div / B
    loss_recon = float(np.mean(0.5 * d1m + 0.5 * d2))
    loss_delta = delta / (B * N)
    loss_dist = dist / B
    terms = np.array([loss_pose, loss_nocs, loss_cd, loss_div, loss_recon,
                      loss_delta, loss_dist], np.float64)
    return np.concatenate([[terms.sum()], terms]).astype(np.float32)


def kernel(**inputs):
    nc = _get_program()
    in_maps = _make_in_maps(**inputs)
    res = run_bass_kernel_spmd(nc, in_maps, list(range(NCORES)))
    return _combine(res.results, inputs["pc_mask"])
